# revision 17
# baseline (speedup 1.0000x reference)
"""Trainium2 Bass kernel for nn_CapsShapeLayer (capsule dynamic routing).

Reference computation:
    u_hat[b,r,c,o] = sum_i W[r,c,o,i] * x[b,r,i]        (151 MB if materialized)
    3 routing iterations:
        c = softmax(b_logits, axis=r)
        s[b,c,o] = sum_r c[r,c] * u_hat[b,r,c,o]
        v = squash(s)                                    (elementwise)
        b_logits += mean_b <u_hat[b,r,c,:], v[b,c,:]>

Kernel strategy (u_hat is never materialized):
  * Shard R=1152 across the 8 cores (144 routes each; K_local = 144*8 = 1152
    = 9 partition tiles of 128 in the fused (r,i) contraction dim).
  * s~[b,(c,o)] = sum_{(r,i)} exp(b)[r,c] * Wt[(r,i),(c,o)] * Xt[(r,i),b]
    -- a K=1152 matmul per core with the softmax divide deferred past the
    cross-core reduction:  s = s~ / S,  S[c] = global sum_r exp(b[r,c]).
  * One AllReduce per iteration carries both the s~ partial [128,256] and
    the 16 exp-sums (128.1 KB payload, 3 collectives total).
  * squash via  v = s|s| / (1+s^2) = s~|s~| / (S^2 + s~^2), with the
    reciprocal computed as exp(-ln(.)) so ScalarE stays on one table set.
  * agreement: a[r,c] = sum_{o,i} W * G with G = Xb^T @ (v/B) -- a K=128
    matmul per (r,i)-tile, then an elementwise W*G and an 8->1 partition
    reduction done as a 0/1-matrix matmul (col-tiled into the right rows).
  * b_logits stays r-sharded per core; iteration 0 (b=0 -> uniform c) skips
    the exp/scale work entirely.
"""

import sys

for _p in ("/opt/trn_rl_repo",):
    if _p not in sys.path:
        sys.path.insert(0, _p)

import numpy as np

import concourse.bass as bass
import concourse.bacc as bacc
import concourse.mybir as mybir
import concourse.tile as tile
from concourse.bass_utils import run_bass_kernel_spmd

F32 = mybir.dt.float32
AX = mybir.AxisListType
ALU = mybir.AluOpType
ACT = mybir.ActivationFunctionType

B = 128          # batch
R = 1152         # routes (input capsules)
C = 16           # output capsules
O = 16           # output capsule dim
I = 8            # input capsule dim
CO = C * O       # 256
NCORES = 8
RS = R // NCORES          # 144 local routes
KL = RS * I               # 1152 local contraction
KT = KL // 128            # 9 K-tiles of 128
NITER = 3
S0 = float(R)             # global softmax denom at iteration 0 (b == 0)
LOG_B = float(np.log(B))

_CACHED = None


def _make_consts():
    """Constant matrices packed into one [128, 193] f32 input.

    cols 0:32    RED_even[p, m] = 1 if m == p//8          (i-reduction, even tile)
    cols 32:64   RED_odd [p, m] = 1 if m == 16 + p//8     (i-reduction, odd tile)
    col  64      ones                                      (partition sum)
    cols 65:193  REP[r', p] = 1 if r' == p//8  (rows 0:16) (c -> (r,i) replicate)
    """
    cst = np.zeros((128, 193), np.float32)
    p = np.arange(128)
    cst[p, p // 8] = 1.0                     # RED_even
    cst[p, 32 + 16 + p // 8] = 1.0           # RED_odd
    cst[:, 64] = 1.0                         # ones
    cst[p // 8, 65 + p] = 1.0                # REP (row r'=p//8, col p)
    # RP[r_loc, 128*t + p] = 1 iff r_loc == 16*t + p//8   (PE operands must
    # sit at base partition 0, so each K-tile gets its own row-select matrix)
    rp = np.zeros((128, 8 * 128), np.float32)
    for t in range(8):
        rp[16 * t + p // 8, 128 * t + p] = 1.0
    return cst, rp


def _prep_inputs(x, W):
    """Host-side shard + relayout. Returns list of 8 per-core input dicts."""
    x = np.ascontiguousarray(np.asarray(x, dtype=np.float32))
    W = np.ascontiguousarray(np.asarray(W, dtype=np.float32))
    Wr = W.reshape(R, C, O, I)
    cst, rp = _make_consts()
    in_maps = []
    for k in range(NCORES):
        sh = slice(k * RS, (k + 1) * RS)
        # Wt[(r,i),(c,o)] tile-interleaved to [p, t, co]
        wt = Wr[sh].transpose(0, 3, 1, 2).reshape(KL, CO)
        wt = np.ascontiguousarray(wt.reshape(KT, 128, CO).transpose(1, 0, 2))
        # Xt[(r,i), b] tile-interleaved to [p, t, b]
        xt = x[:, sh, :].transpose(1, 2, 0).reshape(KL, B)
        xt = np.ascontiguousarray(xt.reshape(KT, 128, B).transpose(1, 0, 2))
        # Xb[b, (r,i)] natural
        xb = np.ascontiguousarray(x[:, sh, :].reshape(B, KL))
        in_maps.append(
            {"wt_in": wt, "xt_in": xt, "xb_in": xb, "cst_in": cst, "rp_in": rp}
        )
    return in_maps


def _build_nc():
    nc = bacc.Bacc(
        "TRN2",
        target_bir_lowering=False,
        debug=False,
        num_devices=NCORES,
    )
    wt_d = nc.dram_tensor("wt_in", [128, KT, CO], F32, kind="ExternalInput")
    xt_d = nc.dram_tensor("xt_in", [128, KT, B], F32, kind="ExternalInput")
    xb_d = nc.dram_tensor("xb_in", [B, KL], F32, kind="ExternalInput")
    cst_d = nc.dram_tensor("cst_in", [128, 193], F32, kind="ExternalInput")
    rp_d = nc.dram_tensor("rp_in", [128, 8 * 128], F32, kind="ExternalInput")
    v_d = nc.dram_tensor("v_out", [B, CO], F32, kind="ExternalOutput")

    rg = [list(range(NCORES))]

    with tile.TileContext(nc) as tc:
        with (
            tc.tile_pool(name="persist", bufs=1) as pp,
            tc.tile_pool(name="work", bufs=2) as wp,
            tc.tile_pool(name="ps_s", bufs=1, space="PSUM") as pool_ps_s,
            tc.tile_pool(name="ps_g", bufs=2, space="PSUM") as pool_ps_g,
            tc.tile_pool(name="ps_small", bufs=2, space="PSUM") as pool_ps_small,
            tc.tile_pool(name="ps_a", bufs=1, space="PSUM") as pool_ps_a,
            tc.tile_pool(name="dram", bufs=1, space="DRAM") as dp,
        ):
            # ---- persistent SBUF state ----
            wt_sb = pp.tile([128, KT, CO], F32, name="wt_sb")
            wc_sb = pp.tile([128, KT, CO], F32, name="wc_sb")
            xt_sb = pp.tile([128, KT, B], F32, name="xt_sb")
            xb_sb = pp.tile([B, KL], F32, name="xb_sb")
            cst_sb = pp.tile([128, 193], F32, name="cst_sb")
            rp_sb = pp.tile([128, 8 * 128], F32, name="rp_sb")
            b_sb = pp.tile([128, 2 * C], F32, name="b_sb")
            nlb_sb = pp.tile([128, 1], F32, name="nlb_sb")
            nc.vector.memset(nlb_sb[:], -LOG_B)

            nc.sync.dma_start(wt_sb[:], wt_d[:])
            nc.sync.dma_start(xt_sb[:], xt_d[:])
            nc.sync.dma_start(xb_sb[:], xb_d[:])
            nc.sync.dma_start(cst_sb[:], cst_d[:])
            nc.sync.dma_start(rp_sb[:], rp_d[:])
            nc.vector.memset(b_sb[:], 0.0)

            red_even = cst_sb[:, 0:32]
            red_odd = cst_sb[:, 32:64]
            red16 = cst_sb[:, 0:16]
            ones_col = cst_sb[:, 64:65]
            rep = cst_sb[0:16, 65:193]



            for it in range(NITER):
                first, last = it == 0, it == NITER - 1

                if first:
                    wmm = wt_sb
                else:
                    # eb = exp(b) for both r-blocks in one ACT op
                    eb = wp.tile([128, 2 * C], F32, name="eb", tag="eb")
                    nc.scalar.activation(eb[:], b_sb[:], ACT.Exp)
                    # local softmax denominator S_loc[c] = sum_r eb[r, c]
                    s_ps = pool_ps_small.tile([1, C], F32, name="s_ps", tag="sp")
                    nc.tensor.matmul(
                        s_ps[:], ones_col[0:128, :], eb[:, 0:C],
                        start=True, stop=False,
                    )
                    nc.tensor.matmul(
                        s_ps[:], ones_col[0:16, :], eb[0:16, C : 2 * C],
                        start=False, stop=True,
                    )
                    # c_part[(r,i), c] = eb[r(p), c] for all 9 K-tiles
                    c_all = pool_ps_small.tile([128, KT * C], F32, name="c_all", tag="sp")
                    for t in range(8):
                        nc.tensor.matmul(
                            c_all[:, C * t : C * (t + 1)],
                            rp_sb[:, 128 * t : 128 * (t + 1)],
                            eb[:, 0:C],
                            start=True, stop=True,
                        )
                    nc.tensor.matmul(
                        c_all[:, C * 8 : C * 9], rep, eb[0:16, C : 2 * C],
                        start=True, stop=True,
                    )
                    # Wc = Wt * c_part   (c broadcast over o), one DVE op
                    c_b = c_all[:].rearrange("p (t c) -> p t c", t=KT)[:, :, :, None]
                    nc.vector.tensor_mul(
                        wc_sb[:],
                        wt_sb[:].rearrange("p t (c o) -> p t c o", c=C),
                        c_b.broadcast_to([128, KT, C, O]),
                    )
                    wmm = wc_sb

                # s~ partial: psum[b, co] = sum_t Xt_t^T @ Wc_t
                ps_s = pool_ps_s.tile([B, CO], F32, name="ps_s", tag="s")
                for t in range(KT):
                    nc.tensor.matmul(
                        ps_s[:],
                        xt_sb[:, t, :],
                        wmm[:, t, :],
                        start=(t == 0),
                        stop=(t == KT - 1),
                    )

                # bounce to DRAM (+ S_loc in cols 256:272) and AllReduce
                ew = CO if first else CO + C
                st_sb = wp.tile([128, ew], F32, name="st_sb", tag=f"st{ew}")
                nc.scalar.copy(st_sb[:, 0:CO], ps_s[:])
                if not first:
                    nc.vector.memset(st_sb[:, CO:ew], 0.0)
                    nc.scalar.copy(st_sb[0:1, CO:ew], s_ps[:])
                cc_in = dp.tile([128, ew], F32, name=f"cc_in{it}")
                cc_out = dp.tile([128, ew], F32, name=f"cc_out{it}", addr_space="Shared")
                nc.sync.dma_start(cc_in[:], st_sb[:])
                nc.gpsimd.collective_compute(
                    "AllReduce", ALU.add, replica_groups=rg,
                    ins=[cc_in[:].opt()], outs=[cc_out[:].opt()],
                )
                s_sb = wp.tile([B, CO], F32, name="s_sb", tag="ssb")
                nc.sync.dma_start(s_sb[:], cc_out[:, 0:CO])

                # squash: v = s~|s~| / (S^2 + s~^2), reciprocal via exp(-ln)
                q2 = wp.tile([B, CO], F32, name="q2", tag="q2")
                if first:
                    qt = wp.tile([B, CO], F32, name="qt", tag="qt")
                    nc.vector.tensor_mul(qt[:], s_sb[:], s_sb[:])
                    nc.vector.tensor_scalar_add(q2[:], qt[:], S0 * S0)
                else:
                    sS = wp.tile([128, C], F32, name="sS", tag="sS")
                    nc.sync.dma_start(
                        sS[:], cc_out[0:1, CO:ew].to_broadcast([128, C])
                    )
                    sS2 = wp.tile([128, C], F32, name="sS2", tag="sS2")
                    nc.vector.tensor_mul(sS2[:], sS[:], sS[:])
                    qt = wp.tile([B, CO], F32, name="qt", tag="qt")
                    nc.vector.tensor_mul(qt[:], s_sb[:], s_sb[:])
                    nc.vector.tensor_add(
                        q2[:].rearrange("b (c o) -> b c o", c=C),
                        qt[:].rearrange("b (c o) -> b c o", c=C),
                        sS2[:, :, None].broadcast_to([B, C, O]),
                    )
                lnq = wp.tile([B, CO], F32, name="lnq", tag="lnq")
                nc.scalar.activation(lnq[:], q2[:], ACT.Ln)
                rec = wp.tile([B, CO], F32, name="rec", tag="rec")
                # fold the 1/B of the batch-mean into G's moving operand here
                nc.scalar.activation(
                    rec[:], lnq[:], ACT.Exp,
                    bias=0.0 if last else nlb_sb[:], scale=-1.0,
                )
                ab = wp.tile([B, CO], F32, name="ab", tag="ab")
                nc.scalar.activation(ab[:], s_sb[:], ACT.Abs)
                m = wp.tile([B, CO], F32, name="m", tag="m")
                nc.vector.tensor_mul(m[:], s_sb[:], ab[:])
                vg = wp.tile([B, CO], F32, name="vg", tag="vg")
                nc.vector.tensor_mul(vg[:], m[:], rec[:])

                if last:
                    nc.sync.dma_start(v_d[:], vg[:])
                    continue

                # G = Xb^T @ (v/B) per (r,i)-tile; P = Wt*G; a = RED^T @ P
                p_sb = wp.tile([128, KT, CO], F32, name="p_sb", tag="p_sb")
                # one PSUM bank: [:, 0] holds a for r 0:128, [0:16, 1] for r 128:144
                ps_a = pool_ps_a.tile([128, 2, C, O], F32, name="ps_a", tag="a")
                for c0 in range(0, KT, 4):
                    nt = min(4, KT - c0)
                    ps_g = pool_ps_g.tile([128, 4, CO], F32, name="ps_g", tag="g")
                    for j in range(nt):
                        t = c0 + j
                        nc.tensor.matmul(
                            ps_g[:, j, :],
                            xb_sb[:, 128 * t : 128 * (t + 1)],
                            vg[:],
                            start=True, stop=True,
                        )
                    nc.vector.tensor_mul(
                        p_sb[:, c0 : c0 + nt, :],
                        wt_sb[:, c0 : c0 + nt, :],
                        ps_g[:, 0:nt, :],
                    )
                for t in range(8):
                    T = t // 2
                    nc.tensor.matmul(
                        ps_a[32 * T : 32 * (T + 1), 0],
                        red_even if t % 2 == 0 else red_odd,
                        p_sb[:, t, :],
                        start=(t % 2 == 0),
                        stop=(t % 2 == 1),
                        tile_position=(0, 32 * T),
                    )
                nc.tensor.matmul(
                    ps_a[0:16, 1], red16, p_sb[:, 8, :], start=True, stop=True
                )
                a_sb = wp.tile([128, C], F32, name="a_sb", tag="a_sb")
                nc.vector.tensor_reduce(a_sb[:], ps_a[:, 0], axis=AX.X, op=ALU.add)
                nc.vector.tensor_add(b_sb[:, 0:C], b_sb[:, 0:C], a_sb[:])
                a2_sb = wp.tile([16, C], F32, name="a2_sb", tag="a2_sb")
                nc.vector.tensor_reduce(
                    a2_sb[:], ps_a[0:16, 1], axis=AX.X, op=ALU.add
                )
                nc.vector.tensor_add(
                    b_sb[0:16, C : 2 * C], b_sb[0:16, C : 2 * C], a2_sb[:]
                )

    nc.compile()
    return nc


def _get_nc():
    global _CACHED
    if _CACHED is None:
        _CACHED = _build_nc()
    return _CACHED


def kernel(x, W):
    nc = _get_nc()
    in_maps = _prep_inputs(x, W)
    res = run_bass_kernel_spmd(nc, in_maps, list(range(NCORES)))
    v = np.asarray(res.results[0]["v_out"])
    return v.reshape(B, C, O).astype(np.float32)
